# revision 27
# baseline (speedup 1.0000x reference)
"""Batched KDE kernel for Trainium2 (8 NeuronCores, SPMD).

Problem: out[b, n] = sum_m exp(-||Xq[b,n] - Xf[b,m]||^2 / bw[b])
  with Silverman bandwidth bw[b] from Xf; b=4, n=m=4096, d=32.

Sharding: data-parallel over batch b (4 batches x 2 shards of query rows
= 8 cores). Each core handles n_shard=2048 query rows against the full
m=4096 fit set of its batch.

Device algorithm (per core), raw Bass with manual semaphores:
  The exp argument (2*q.f - |q|^2 - |f|^2)/bw is produced by ONE K=100
  bf16 matmul per 512-col psum bank (f32 values x = x1+x2+O(2^-16)):
    lhsT rows = [q1 q1 q2 | -1 -1 b1 b2]   (q pieces carry 2/bw)
    rhs  rows = [f1 f2 f1 | s1 s2 1  1]    (s = |f|^2/bw, b = -|q|^2/bw)
  capturing q1f1+q1f2+q2f1 (dropped q2f2 ~ 2e-3 on the argument,
  rel err ~4e-4 on the output -- tolerance is 2e-2).
  ScalarE activation computes exp(psum) with a fused per-partition
  accumulate (accum_out) -> the sum over m. ACT is the bottleneck
  (~1.94us per 2048-col group, 32 groups ~ 62us); PE at 1 matmul/chunk
  (~1us/group) runs far ahead, so the schedule is ACT-dense:
    - double-buffered psum (ps0/ps1, 4 banks each); ACT group g reads
      ps[g%2] while PE fills ps[(g+1)%2]; PE waits s_act >= g-1 before
      reusing a psum half
    - all constants (-1 rows, bias rows) are baked into the one dram
      blob; no memsets, no meta transfer, bias/scale are immediates
    - DMA is ordered by first use and split across both HWDGE queues:
      the first-group gate (lhsT t0 + rhs c0/c1 on sync, rhs c2/c3 on
      scalar) leads both in-order rings in parallel, and the bulk (lhsT
      t1-15, rhs chunks 4-7) is held behind explicit waits on the gate
      so it cannot crowd it out of the shared HBM port
    - s_pe for group g is released on its 3rd of 4 matmuls: the ACT
      reads bank 3 >=1.5us after waking while PE writes it ~0.25us
      after the release
    - PE warmup matmuls on garbage SBUF start immediately so the HAM
      clock gate is released before the first real group
    - group 0 runs as three pieces (512+512+1024 cols) so the first
      exp needs only lhsT t0 + rhs c0 (~130 KB) instead of the whole
      first-group gate
    - groups run h=0 tiles 0..15 then h=1 tiles 15..0 (reversed), so
      the final acc->res reduction is split in three and only a 3-slot
      reduce sits after the last exp group
  NOTE: engines run in relaxed ordering mode -- any same-engine RAW
  needs an explicit semaphore between producer and consumer.
Host does sharding/layout/packing plus the 4 scalar bandwidth values and
query norms (global quantile needs a sort; both are O(input) prep).
"""

import numpy as np

B, N, M, D = 4, 4096, 4096, 32
NCORES = 8
SHARDS_PER_BATCH = NCORES // B  # 2
NSHARD = N // SHARDS_PER_BATCH  # 2048
NT = NSHARD // 128  # 16 n-tiles per core
MCHUNK = 512  # matmul free-dim chunk (one psum bank)
ACT_FD = 2048  # activation free dim (4 psum banks)
NG = NT * (M // ACT_FD)  # 32 matmul/exp groups
KROWS = 100  # lhsT/rhs contraction rows

# blob column offsets (bf16 cols), ordered by first use
OFF_L0 = 0  # lhsT tile 0 (128)
OFF_R0 = 128  # rhs chunks 0-3 (2048)
OFF_LT = 2176  # lhsT tiles 1..15 (1920)
OFF_R1 = 4096  # rhs chunks 4-7 (2048)
BLOB_W = 6144

_cached = {}


def _l_off(t):
    return OFF_L0 if t == 0 else OFF_LT + (t - 1) * 128


def _r_off(c):  # c = m-col / 512, 0..7
    return OFF_R0 + c * MCHUNK if c < 4 else OFF_R1 + (c - 4) * MCHUNK


def _build_program():
    import concourse.bass as bass
    import concourse.mybir as mybir
    from contextlib import ExitStack

    nc = bass.Bass()
    f32 = mybir.dt.float32
    bf16 = mybir.dt.bfloat16

    blob = nc.declare_dram_parameter("blob", [128, BLOB_W], bf16, isOutput=False)
    res = nc.declare_dram_parameter("res", [128, NT], f32, isOutput=True)

    # pacer copy length: anchor (READ_ACC complete ~ T+240) + vec wake
    # (~100) + copy + inc/PE wake (~150) must land the fill start in
    # [T+2204, T+2839] where T = start of the activation being chased
    PACE_COLS = 5336  # copy ~1.45us; the DVE pipe DRAIN costs about a
    # full copy again, so copy must stay well under half the 3.65us
    # activation period or the pacer chain backs up

    with ExitStack() as ctx:
        msb = ctx.enter_context(nc.sbuf_tensor([128, BLOB_W], bf16))
        escr0 = ctx.enter_context(nc.sbuf_tensor([128, ACT_FD], bf16))
        escr1 = ctx.enter_context(nc.sbuf_tensor([128, ACT_FD], bf16))
        escr4 = ctx.enter_context(nc.sbuf_tensor([128, 2 * ACT_FD], bf16))
        # acc slots: group0 pieces 0,1,2; t0h1 3; tiles 1-3 -> 2+2t+h
        acc = ctx.enter_context(nc.sbuf_tensor([128, 10], f32))
        res_sb = ctx.enter_context(nc.sbuf_tensor([128, NT], f32))
        warmT = ctx.enter_context(nc.sbuf_tensor([128, 1], f32))
        wscr = ctx.enter_context(nc.sbuf_tensor([128, 640], bf16))
        vd1 = ctx.enter_context(nc.sbuf_tensor([128, PACE_COLS], bf16))
        vd2 = ctx.enter_context(nc.sbuf_tensor([128, PACE_COLS], bf16))
        psA = ctx.enter_context(nc.psum_tensor("psA", [128, 2 * ACT_FD], f32))

        s_l0 = ctx.enter_context(nc.semaphore("s_l0"))
        s_r0a = ctx.enter_context(nc.semaphore("s_r0a"))
        s_r0b = ctx.enter_context(nc.semaphore("s_r0b"))
        s_r1a = ctx.enter_context(nc.semaphore("s_r1a"))
        s_r1b = ctx.enter_context(nc.semaphore("s_r1b"))
        s_lta = ctx.enter_context(nc.semaphore("s_lta"))
        s_ltb = ctx.enter_context(nc.semaphore("s_ltb"))
        s_pe = ctx.enter_context(nc.semaphore("s_pe"))
        s_act = ctx.enter_context(nc.semaphore("s_act"))
        s_vd = ctx.enter_context(nc.semaphore("s_vd"))
        s_v3 = ctx.enter_context(nc.semaphore("s_v3"))
        sem_out = ctx.enter_context(nc.semaphore("sem_out"))
        block = ctx.enter_context(nc.Block())

        # ACT step table: (psum src, elementwise out, accum out) with
        # s_pe >= 1-based index gating each step.
        # idx 1-3: group-0 pieces (512+512+1024, tile 0 h0)
        # idx 4-6: tiles 1-3 h0 (FD 2048, alternating psum halves)
        # idx 7-18: tiles 4-15 full rows (FD 4096, accum -> res directly)
        # idx 19-22: tiles 3,2,1,0 h1 (FD 2048)
        lo, hi = psA[:, 0:ACT_FD], psA[:, ACT_FD : 2 * ACT_FD]
        steps = [
            (psA[:, 0:512], escr0[:, 0:512], acc[:, 0:1]),
            (psA[:, 512:1024], escr0[:, 0:512], acc[:, 1:2]),
            (psA[:, 1024:2048], escr0[:, 0:1024], acc[:, 2:3]),
            (hi, escr1[:], acc[:, 4:5]),
            (lo, escr0[:], acc[:, 6:7]),
            (hi, escr1[:], acc[:, 8:9]),
        ]
        for t in range(4, NT):
            steps.append((psA[:], escr4[:], res_sb[:, t : t + 1]))
        steps += [
            (lo, escr0[:], acc[:, 9:10]),
            (hi, escr1[:], acc[:, 7:8]),
            (lo, escr0[:], acc[:, 5:6]),
            (hi, escr1[:], acc[:, 3:4]),
        ]

        @block.sync
        def _(sync):
            # first-group gate leads both rings; bulk held behind it
            sync.dma_start(
                msb[0:KROWS, OFF_L0 : OFF_R0 + 512],
                blob[0:KROWS, OFF_L0 : OFF_R0 + 512],
            ).then_inc(s_l0, 16)
            sync.wait_ge(s_r0b, 16)
            sync.wait_ge(s_lta, 16)
            sync.dma_start(
                msb[0:KROWS, OFF_LT + 384 : OFF_R1],
                blob[0:KROWS, OFF_LT + 384 : OFF_R1],
            ).then_inc(s_ltb, 16)
            sync.dma_start(
                msb[0:KROWS, OFF_R1 : OFF_R1 + 1024],
                blob[0:KROWS, OFF_R1 : OFF_R1 + 1024],
            ).then_inc(s_r1a, 16)
            sync.dma_start(
                msb[0:KROWS, OFF_R1 + 1024 : BLOB_W],
                blob[0:KROWS, OFF_R1 + 1024 : BLOB_W],
            ).then_inc(s_r1b, 16)
            # tiles 4-15 land in res_sb straight from the accumulator
            # reads; step 18's READ_ACC fires s_act >= 18
            sync.wait_ge(s_act, 18)
            sync.dma_start(res[:, 4:16], res_sb[:, 4:16]).then_inc(sem_out, 16)
            sync.wait_ge(s_v3, 1)
            # no completion wait: the NEFF teardown drains the DMA queue
            sync.dma_start(res[:, 0:4], res_sb[:, 0:4]).then_inc(sem_out, 16)

        @block.vector
        def _(vector):
            # pacers: the fill for tile t (t=5..15) chases the FD-4096
            # read of tile t-1; t3h1's fill chases tile 15's read. Each
            # pacer anchors on the chased activation's start (its
            # predecessor's READ_ACC completion) and burns a
            # cycle-accurate DVE copy before releasing the PE.
            for k, a in enumerate(list(range(6, 17)) + [17]):
                vector.wait_ge(s_act, a)
                nc.vector.tensor_copy(vd2[:], vd1[:]).then_inc(s_vd, 1)
            # tail reductions for the h-split tiles, earliest first
            vector.wait_ge(s_act, 19)
            nc.vector.tensor_reduce(
                res_sb[:, 3:4],
                acc[:, 8:10].rearrange("p (t h) -> p t h", h=2),
                axis=mybir.AxisListType.X,
                op=mybir.AluOpType.add,
            )
            vector.wait_ge(s_act, 20)
            nc.vector.tensor_reduce(
                res_sb[:, 2:3],
                acc[:, 6:8].rearrange("p (t h) -> p t h", h=2),
                axis=mybir.AxisListType.X,
                op=mybir.AluOpType.add,
            )
            vector.wait_ge(s_act, 21)
            nc.vector.tensor_reduce(
                res_sb[:, 1:2],
                acc[:, 4:6].rearrange("p (t h) -> p t h", h=2),
                axis=mybir.AxisListType.X,
                op=mybir.AluOpType.add,
            )
            vector.wait_ge(s_act, 22)
            nc.vector.tensor_reduce(
                res_sb[:, 0:1],
                acc[:, 0:4].rearrange("p (t h) -> p t h", h=4),
                axis=mybir.AxisListType.X,
                op=mybir.AluOpType.add,
            ).then_inc(s_v3, 1)

        @block.scalar
        def _(scalar):
            # fire the exp table-set load FIRST: it takes ~1.6us and
            # otherwise gates the first exp from behind the DMA issues
            # (the psum data is ready before the table when it is last)
            nc.scalar.activation(
                warmT[:], warmT[:], mybir.ActivationFunctionType.Exp,
                bias=res_sb[:, 0:1],
            )
            # second HWDGE queue: rhs c1, c2/c3, then lhsT tiles 1-3
            scalar.dma_start(
                msb[0:KROWS, OFF_R0 + 512 : OFF_R0 + 1024],
                blob[0:KROWS, OFF_R0 + 512 : OFF_R0 + 1024],
            ).then_inc(s_r0a, 16)
            scalar.dma_start(
                msb[0:KROWS, OFF_R0 + 1024 : OFF_LT],
                blob[0:KROWS, OFF_R0 + 1024 : OFF_LT],
            ).then_inc(s_r0b, 16)
            scalar.dma_start(
                msb[0:KROWS, OFF_LT : OFF_LT + 384],
                blob[0:KROWS, OFF_LT : OFF_LT + 384],
            ).then_inc(s_lta, 16)
            for k, (src_ap, out_ap, acc_ap) in enumerate(steps):
                scalar.wait_ge(s_pe, k + 1)
                nc.scalar.add_instruction(
                    mybir.InstActivation(
                        name=nc.get_next_instruction_name(),
                        func=mybir.ActivationFunctionType.Exp,
                        ins=[
                            nc.scalar.lower_ap(src_ap),
                            mybir.ImmediateValue(
                                dtype=mybir.dt.float32, value=0.0
                            ),
                            mybir.ImmediateValue(
                                dtype=mybir.dt.float32, value=1.0
                            ),
                            mybir.ImmediateValue(
                                dtype=mybir.dt.float32, value=0.0
                            ),
                        ],
                        outs=[
                            nc.scalar.lower_ap(out_ap),
                            nc.scalar.lower_ap(acc_ap),
                        ],
                    )
                ).then_inc(s_act, 1)

        @block.tensor
        def _(tensor):
            # warm the PE clock (HAM) with dummy matmuls on garbage SBUF;
            # they run at half clock (~427ns each) so keep them few: the
            # first-piece matmuls have 2-3x slack vs their activations
            # even if the clock is still ramping
            for _w in range(4):
                nc.tensor.matmul(
                    psA[:, 0:MCHUNK],
                    wscr[:, 0:128],
                    wscr[:, 128:640],
                    start=True,
                    stop=True,
                )

            def fill(t, chunks, bank0, rel_on, sub=None):
                # one matmul per 512-col bank; inc s_pe on matmul rel_on
                lh = msb[0:KROWS, _l_off(t) : _l_off(t) + 128]
                for j, c in enumerate(chunks):
                    if sub:
                        sub(j)
                    mm = nc.tensor.matmul(
                        psA[:, (bank0 + j) * MCHUNK : (bank0 + j + 1) * MCHUNK],
                        lh,
                        msb[0:KROWS, _r_off(c) : _r_off(c) + MCHUNK],
                        start=True,
                        stop=True,
                    )
                    if j == rel_on:
                        mm.then_inc(s_pe, 1)

            # group-0 pieces: release per piece as its banks complete
            tensor.wait_ge(s_l0, 16)

            def g0_sub(j):
                if j == 1:
                    tensor.wait_ge(s_r0a, 16)
                if j == 2:
                    tensor.wait_ge(s_r0b, 16)

            lh0 = msb[0:KROWS, _l_off(0) : _l_off(0) + 128]
            for j in range(4):
                g0_sub(j)
                mm = nc.tensor.matmul(
                    psA[:, j * MCHUNK : (j + 1) * MCHUNK],
                    lh0,
                    msb[0:KROWS, _r_off(j) : _r_off(j) + MCHUNK],
                    start=True,
                    stop=True,
                )
                if j in (0, 1, 3):
                    mm.then_inc(s_pe, 1)

            # tiles 1-3 h0: classic double-buffered halves
            tensor.wait_ge(s_lta, 16)
            fill(1, range(4), 4, 2)
            tensor.wait_ge(s_act, 3)
            fill(2, range(4), 0, 2)
            tensor.wait_ge(s_act, 4)
            fill(3, range(4), 4, 2)
            # tile 4: pre-fill the low half during t3h0's read, then the
            # high half once t3h0's activation is done -- its FD-4096
            # read only touches the high banks ~1.9us after waking
            tensor.wait_ge(s_act, 5)
            tensor.wait_ge(s_ltb, 16)
            fill(4, range(4), 0, 2)
            tensor.wait_ge(s_act, 6)
            tensor.wait_ge(s_r1a, 16)
            tensor.wait_ge(s_r1b, 16)
            fill(4, range(4, 8), 4, None)
            # tiles 5-15: banks 0-6 are paced by the DVE behind the
            # previous tile's sweeping FD-4096 read (the shorter copy
            # keeps the pacer chain under the activation period); bank 7
            # waits until the chased activation has provably started
            # (its predecessor's READ_ACC completion), which is later
            # than that read's last touch of bank 7
            for k, t in enumerate(range(5, NT)):
                tensor.wait_ge(s_vd, k + 1)
                fill(t, range(6), 0, 2)
                tensor.wait_ge(s_act, t + 2)
                fill(t, [6, 7], 6, None)
            # h1 halves of tiles 3..0; the first chases tile 15's read,
            # the rest are semaphore-clean double-buffering
            tensor.wait_ge(s_vd, 12)
            fill(3, range(4, 8), 0, 2)
            tensor.wait_ge(s_act, 18)
            fill(2, range(4, 8), 4, 2)
            tensor.wait_ge(s_act, 19)
            fill(1, range(4, 8), 0, 2)
            tensor.wait_ge(s_act, 20)
            fill(0, range(4, 8), 4, 2)

    return nc


def _bf16_split2(x):
    import ml_dtypes

    bf = ml_dtypes.bfloat16
    x = x.astype(np.float32)
    p1 = x.astype(bf)
    p2 = (x - p1.astype(np.float32)).astype(bf)
    return p1, p2


def _bandwidth_np(X_fit):
    # mirror of reference._bandwidth (Silverman-style)
    b, n, d = X_fit.shape
    flat = np.asarray(X_fit, dtype=np.float64).reshape(-1)
    q = np.quantile(flat, 0.75) - np.quantile(flat, 0.25)
    std = np.std(np.asarray(X_fit, dtype=np.float64).reshape(b, -1), axis=1, ddof=1)
    return (0.9 * np.minimum(std, q / 1.34) / (n**0.2)).astype(np.float32)


def _host_prep(X_query, X_fit):
    import ml_dtypes

    bf = ml_dtypes.bfloat16
    X_query = np.asarray(X_query, dtype=np.float32)
    X_fit = np.asarray(X_fit, dtype=np.float32)
    bw = _bandwidth_np(X_fit)  # [B]

    in_maps = []
    for c in range(NCORES):
        b = c // SHARDS_PER_BATCH
        s = c % SHARDS_PER_BATCH
        XQ = X_query[b, s * NSHARD : (s + 1) * NSHARD]  # [2048, 32]
        XF = X_fit[b]  # [4096, 32]
        inv = np.float64(1.0) / np.float64(bw[b])

        # permuted queries: tile t / partition p handles query row p*NT+t;
        # 2/bw rides in the q pieces so psum IS the exp argument
        XQp = XQ.reshape(128, NT, D).transpose(1, 0, 2).reshape(NSHARD, D)
        Q = (2.0 * inv * XQp.T.astype(np.float64)).astype(np.float32)  # [32, 2048]
        q1, q2 = _bf16_split2(Q)
        FT = np.ascontiguousarray(XF.T.astype(np.float32))  # [32, 4096]
        f1, f2 = _bf16_split2(FT)
        sv = (FT.astype(np.float64) ** 2 * inv).sum(0).astype(np.float32)  # [4096]
        s1, s2 = _bf16_split2(sv)
        nx2 = (XQp.astype(np.float64) ** 2).sum(1)  # [2048] tile-major
        br = ((-nx2) * inv).astype(np.float32)
        b1, b2 = _bf16_split2(br)

        rhs = np.empty((KROWS, M), dtype=bf)
        rhs[0:32] = f1
        rhs[32:64] = f2
        rhs[64:96] = f1
        rhs[96] = s1
        rhs[97] = s2
        rhs[98:100] = np.ones((2, M), dtype=bf)

        blob = np.zeros((128, BLOB_W), dtype=bf)
        for t in range(NT):
            lo = _l_off(t)
            cols = slice(t * 128, (t + 1) * 128)
            blob[0:32, lo : lo + 128] = q1[:, cols]
            blob[32:64, lo : lo + 128] = q1[:, cols]
            blob[64:96, lo : lo + 128] = q2[:, cols]
            blob[96:98, lo : lo + 128] = -1.0
            blob[98, lo : lo + 128] = b1[cols]
            blob[99, lo : lo + 128] = b2[cols]
        blob[0:KROWS, OFF_R0:OFF_LT] = rhs[:, 0:2048]
        blob[0:KROWS, OFF_R1:BLOB_W] = rhs[:, 2048:4096]

        in_maps.append({"blob": blob})
    return in_maps


def _gather(results):
    out = np.empty((B, N), dtype=np.float32)
    for c in range(NCORES):
        b = c // SHARDS_PER_BATCH
        s = c % SHARDS_PER_BATCH
        res = np.asarray(results[c]["res"], dtype=np.float32)  # [128, 16]
        out[b, s * NSHARD : (s + 1) * NSHARD] = res.reshape(NSHARD)
    return out


def kernel(X_query, X_fit):
    from concourse.bass_utils import run_bass_kernel_spmd

    if "nc" not in _cached:
        _cached["nc"] = _build_program()
    nc = _cached["nc"]
    in_maps = _host_prep(X_query, X_fit)
    out = run_bass_kernel_spmd(nc, in_maps, list(range(NCORES)))
    return _gather(out.results)


# revision 29
# speedup vs baseline: 1.0084x; 1.0084x over previous
"""Batched KDE kernel for Trainium2 (8 NeuronCores, SPMD).

Problem: out[b, n] = sum_m exp(-||Xq[b,n] - Xf[b,m]||^2 / bw[b])
  with Silverman bandwidth bw[b] from Xf; b=4, n=m=4096, d=32.

Sharding: data-parallel over batch b (4 batches x 2 shards of query rows
= 8 cores). Each core handles n_shard=2048 query rows against the full
m=4096 fit set of its batch.

Device algorithm (per core), raw Bass with manual semaphores:
  The exp argument (2*q.f - |q|^2 - |f|^2)/bw is produced by ONE K=100
  bf16 matmul per 512-col psum bank (f32 values x = x1+x2+O(2^-16)):
    lhsT rows = [q1 q1 q2 | -1 -1 b1 b2]   (q pieces carry 2/bw)
    rhs  rows = [f1 f2 f1 | s1 s2 1  1]    (s = |f|^2/bw, b = -|q|^2/bw)
  capturing q1f1+q1f2+q2f1 (dropped q2f2 ~ 2e-3 on the argument,
  rel err ~4e-4 on the output -- tolerance is 2e-2).
  ScalarE activation computes exp(psum) with a fused per-partition
  accumulate (accum_out) -> the sum over m. ACT is the bottleneck
  (~1.94us per 2048-col group, 32 groups ~ 62us); PE at 1 matmul/chunk
  (~1us/group) runs far ahead, so the schedule is ACT-dense:
    - double-buffered psum (ps0/ps1, 4 banks each); ACT group g reads
      ps[g%2] while PE fills ps[(g+1)%2]; PE waits s_act >= g-1 before
      reusing a psum half
    - all constants (-1 rows, bias rows) are baked into the one dram
      blob; no memsets, no meta transfer, bias/scale are immediates
    - DMA is ordered by first use and split across both HWDGE queues:
      the first-group gate (lhsT t0 + rhs c0/c1 on sync, rhs c2/c3 on
      scalar) leads both in-order rings in parallel, and the bulk (lhsT
      t1-15, rhs chunks 4-7) is held behind explicit waits on the gate
      so it cannot crowd it out of the shared HBM port
    - s_pe for group g is released on its 3rd of 4 matmuls: the ACT
      reads bank 3 >=1.5us after waking while PE writes it ~0.25us
      after the release
    - PE warmup matmuls on garbage SBUF start immediately so the HAM
      clock gate is released before the first real group
    - group 0 runs as three pieces (512+512+1024 cols) so the first
      exp needs only lhsT t0 + rhs c0 (~130 KB) instead of the whole
      first-group gate
    - groups run h=0 tiles 0..15 then h=1 tiles 15..0 (reversed), so
      the final acc->res reduction is split in three and only a 3-slot
      reduce sits after the last exp group
  NOTE: engines run in relaxed ordering mode -- any same-engine RAW
  needs an explicit semaphore between producer and consumer.
Host does sharding/layout/packing plus the 4 scalar bandwidth values and
query norms (global quantile needs a sort; both are O(input) prep).
"""

import numpy as np

B, N, M, D = 4, 4096, 4096, 32
NCORES = 8
SHARDS_PER_BATCH = NCORES // B  # 2
NSHARD = N // SHARDS_PER_BATCH  # 2048
NT = NSHARD // 128  # 16 n-tiles per core
MCHUNK = 512  # matmul free-dim chunk (one psum bank)
ACT_FD = 2048  # activation free dim (4 psum banks)
NG = NT * (M // ACT_FD)  # 32 matmul/exp groups
KROWS = 100  # lhsT/rhs contraction rows

# blob column offsets (bf16 cols), ordered by first use
OFF_L0 = 0  # lhsT tile 0 (128)
OFF_R0 = 128  # rhs chunks 0-3 (2048)
OFF_LT = 2176  # lhsT tiles 1..15 (1920)
OFF_R1 = 4096  # rhs chunks 4-7 (2048)
BLOB_W = 6144

_cached = {}


def _l_off(t):
    return OFF_L0 if t == 0 else OFF_LT + (t - 1) * 128


def _r_off(c):  # c = m-col / 512, 0..7
    return OFF_R0 + c * MCHUNK if c < 4 else OFF_R1 + (c - 4) * MCHUNK


def _build_program():
    import concourse.bass as bass
    import concourse.mybir as mybir
    from contextlib import ExitStack

    nc = bass.Bass()
    f32 = mybir.dt.float32
    bf16 = mybir.dt.bfloat16

    blob = nc.declare_dram_parameter("blob", [128, BLOB_W], bf16, isOutput=False)
    res = nc.declare_dram_parameter("res", [128, NT], f32, isOutput=True)

    # pacer copy length: anchor (READ_ACC complete ~ T+240) + vec wake
    # (~100) + copy + inc/PE wake (~150) must land the fill start in
    # [T+2204, T+2839] where T = start of the activation being chased
    PACE_COLS = 5336  # copy ~1.45us; the DVE pipe DRAIN costs about a
    # full copy again, so copy must stay well under half the 3.65us
    # activation period or the pacer chain backs up

    with ExitStack() as ctx:
        msb = ctx.enter_context(nc.sbuf_tensor([128, BLOB_W], bf16))
        escr0 = ctx.enter_context(nc.sbuf_tensor([128, ACT_FD], bf16))
        escr1 = ctx.enter_context(nc.sbuf_tensor([128, ACT_FD], bf16))
        escr4 = ctx.enter_context(nc.sbuf_tensor([128, 2 * ACT_FD], bf16))
        # acc slots: group0 pieces 0,1,2; t0h1 3; tiles 1-3 -> 2+2t+h
        acc = ctx.enter_context(nc.sbuf_tensor([128, 10], f32))
        res_sb = ctx.enter_context(nc.sbuf_tensor([128, NT], f32))
        warmT = ctx.enter_context(nc.sbuf_tensor([128, 1], f32))
        wscr = ctx.enter_context(nc.sbuf_tensor([128, 640], bf16))
        vd1 = ctx.enter_context(nc.sbuf_tensor([128, PACE_COLS], bf16))
        vd2 = ctx.enter_context(nc.sbuf_tensor([128, PACE_COLS], bf16))
        psA = ctx.enter_context(nc.psum_tensor("psA", [128, 2 * ACT_FD], f32))

        s_l0 = ctx.enter_context(nc.semaphore("s_l0"))
        s_r0a = ctx.enter_context(nc.semaphore("s_r0a"))
        s_r0b = ctx.enter_context(nc.semaphore("s_r0b"))
        s_r1a = ctx.enter_context(nc.semaphore("s_r1a"))
        s_r1b = ctx.enter_context(nc.semaphore("s_r1b"))
        s_lta = ctx.enter_context(nc.semaphore("s_lta"))
        s_ltb = ctx.enter_context(nc.semaphore("s_ltb"))
        s_pe = ctx.enter_context(nc.semaphore("s_pe"))
        s_act = ctx.enter_context(nc.semaphore("s_act"))
        s_vd = ctx.enter_context(nc.semaphore("s_vd"))
        s_v3 = ctx.enter_context(nc.semaphore("s_v3"))
        sem_out = ctx.enter_context(nc.semaphore("sem_out"))
        block = ctx.enter_context(nc.Block())

        # ACT step table: (psum src, elementwise out, accum out) with
        # s_pe >= 1-based index gating each step.
        # idx 1-3: group-0 pieces (512+512+1024, tile 0 h0)
        # idx 4-6: tiles 1-3 h0 (FD 2048, alternating psum halves)
        # idx 7-18: tiles 4-15 full rows (FD 4096, accum -> res directly)
        # idx 19-22: tiles 3,2,1,0 h1 (FD 2048)
        lo, hi = psA[:, 0:ACT_FD], psA[:, ACT_FD : 2 * ACT_FD]
        steps = [
            (psA[:, 0:512], escr0[:, 0:512], acc[:, 0:1]),
            (psA[:, 512:1024], escr0[:, 0:512], acc[:, 1:2]),
            (psA[:, 1024:2048], escr0[:, 0:1024], acc[:, 2:3]),
            (hi, escr1[:], acc[:, 4:5]),
            (lo, escr0[:], acc[:, 6:7]),
            (hi, escr1[:], acc[:, 8:9]),
        ]
        for t in range(4, NT):
            steps.append((psA[:], escr4[:], res_sb[:, t : t + 1]))
        steps += [
            (lo, escr0[:], acc[:, 9:10]),
            (hi, escr1[:], acc[:, 7:8]),
            (lo, escr0[:], acc[:, 5:6]),
            (hi, escr1[:], acc[:, 3:4]),
        ]

        @block.sync
        def _(sync):
            # first-group gate leads both rings; bulk held behind it
            sync.dma_start(
                msb[0:KROWS, OFF_L0 : OFF_R0 + 512],
                blob[0:KROWS, OFF_L0 : OFF_R0 + 512],
            ).then_inc(s_l0, 16)
            sync.wait_ge(s_r0b, 16)
            sync.wait_ge(s_lta, 16)
            sync.dma_start(
                msb[0:KROWS, OFF_LT + 384 : OFF_R1],
                blob[0:KROWS, OFF_LT + 384 : OFF_R1],
            ).then_inc(s_ltb, 16)
            sync.dma_start(
                msb[0:KROWS, OFF_R1 : OFF_R1 + 1024],
                blob[0:KROWS, OFF_R1 : OFF_R1 + 1024],
            ).then_inc(s_r1a, 16)
            sync.dma_start(
                msb[0:KROWS, OFF_R1 + 1024 : BLOB_W],
                blob[0:KROWS, OFF_R1 + 1024 : BLOB_W],
            ).then_inc(s_r1b, 16)
            # tiles 4-15 land in res_sb straight from the accumulator
            # reads; step 18's READ_ACC fires s_act >= 18
            sync.wait_ge(s_act, 18)
            sync.dma_start(res[:, 4:16], res_sb[:, 4:16]).then_inc(sem_out, 16)
            sync.wait_ge(s_v3, 1)
            # no completion wait: the NEFF teardown drains the DMA queue
            sync.dma_start(res[:, 0:4], res_sb[:, 0:4]).then_inc(sem_out, 16)

        @block.vector
        def _(vector):
            # pacers: the fill for tile t (t=5..15) chases the FD-4096
            # read of tile t-1; t3h1's fill chases tile 15's read. Each
            # pacer anchors on the chased activation's start (its
            # predecessor's READ_ACC completion) and burns a
            # cycle-accurate DVE copy before releasing the PE.
            for k, a in enumerate(list(range(6, 17)) + [17]):
                vector.wait_ge(s_act, a)
                nc.vector.tensor_copy(vd2[:], vd1[:]).then_inc(s_vd, 1)
            # tail reductions for the h-split tiles, earliest first
            vector.wait_ge(s_act, 19)
            nc.vector.tensor_reduce(
                res_sb[:, 3:4],
                acc[:, 8:10].rearrange("p (t h) -> p t h", h=2),
                axis=mybir.AxisListType.X,
                op=mybir.AluOpType.add,
            )
            vector.wait_ge(s_act, 20)
            nc.vector.tensor_reduce(
                res_sb[:, 2:3],
                acc[:, 6:8].rearrange("p (t h) -> p t h", h=2),
                axis=mybir.AxisListType.X,
                op=mybir.AluOpType.add,
            )
            vector.wait_ge(s_act, 21)
            nc.vector.tensor_reduce(
                res_sb[:, 1:2],
                acc[:, 4:6].rearrange("p (t h) -> p t h", h=2),
                axis=mybir.AxisListType.X,
                op=mybir.AluOpType.add,
            )
            vector.wait_ge(s_act, 22)
            nc.vector.tensor_reduce(
                res_sb[:, 0:1],
                acc[:, 0:4].rearrange("p (t h) -> p t h", h=4),
                axis=mybir.AxisListType.X,
                op=mybir.AluOpType.add,
            ).then_inc(s_v3, 1)

        @block.scalar
        def _(scalar):
            # fire the exp table-set load FIRST: it takes ~1.6us and
            # otherwise gates the first exp from behind the DMA issues
            # (the psum data is ready before the table when it is last)
            nc.scalar.activation(
                warmT[:], warmT[:], mybir.ActivationFunctionType.Exp,
                bias=res_sb[:, 0:1],
            )
            # second HWDGE queue: rhs c1, c2/c3, then lhsT tiles 1-3
            scalar.dma_start(
                msb[0:KROWS, OFF_R0 + 512 : OFF_R0 + 1024],
                blob[0:KROWS, OFF_R0 + 512 : OFF_R0 + 1024],
            ).then_inc(s_r0a, 16)
            scalar.dma_start(
                msb[0:KROWS, OFF_R0 + 1024 : OFF_LT],
                blob[0:KROWS, OFF_R0 + 1024 : OFF_LT],
            ).then_inc(s_r0b, 16)
            scalar.dma_start(
                msb[0:KROWS, OFF_LT : OFF_LT + 384],
                blob[0:KROWS, OFF_LT : OFF_LT + 384],
            ).then_inc(s_lta, 16)
            for k, (src_ap, out_ap, acc_ap) in enumerate(steps):
                scalar.wait_ge(s_pe, k + 1)
                nc.scalar.add_instruction(
                    mybir.InstActivation(
                        name=nc.get_next_instruction_name(),
                        func=mybir.ActivationFunctionType.Exp,
                        ins=[
                            nc.scalar.lower_ap(src_ap),
                            mybir.ImmediateValue(
                                dtype=mybir.dt.float32, value=0.0
                            ),
                            mybir.ImmediateValue(
                                dtype=mybir.dt.float32, value=1.0
                            ),
                            mybir.ImmediateValue(
                                dtype=mybir.dt.float32, value=0.0
                            ),
                        ],
                        outs=[
                            nc.scalar.lower_ap(out_ap),
                            nc.scalar.lower_ap(acc_ap),
                        ],
                    )
                ).then_inc(s_act, 1)

        @block.tensor
        def _(tensor):
            # warm the PE clock (HAM) with dummy matmuls on garbage SBUF;
            # they run at half clock (~427ns each) so keep them few: the
            # first-piece matmuls have 2-3x slack vs their activations
            # even if the clock is still ramping
            for _w in range(7):
                nc.tensor.matmul(
                    psA[:, 0:MCHUNK],
                    wscr[:, 0:128],
                    wscr[:, 128:640],
                    start=True,
                    stop=True,
                )

            def fill(t, chunks, bank0, rel_on, sub=None):
                # one matmul per 512-col bank; inc s_pe on matmul rel_on
                lh = msb[0:KROWS, _l_off(t) : _l_off(t) + 128]
                for j, c in enumerate(chunks):
                    if sub:
                        sub(j)
                    mm = nc.tensor.matmul(
                        psA[:, (bank0 + j) * MCHUNK : (bank0 + j + 1) * MCHUNK],
                        lh,
                        msb[0:KROWS, _r_off(c) : _r_off(c) + MCHUNK],
                        start=True,
                        stop=True,
                    )
                    if j == rel_on:
                        mm.then_inc(s_pe, 1)

            # group-0 pieces: release per piece as its banks complete
            tensor.wait_ge(s_l0, 16)

            def g0_sub(j):
                if j == 1:
                    tensor.wait_ge(s_r0a, 16)
                if j == 2:
                    tensor.wait_ge(s_r0b, 16)

            lh0 = msb[0:KROWS, _l_off(0) : _l_off(0) + 128]
            for j in range(4):
                g0_sub(j)
                mm = nc.tensor.matmul(
                    psA[:, j * MCHUNK : (j + 1) * MCHUNK],
                    lh0,
                    msb[0:KROWS, _r_off(j) : _r_off(j) + MCHUNK],
                    start=True,
                    stop=True,
                )
                if j in (0, 1, 3):
                    mm.then_inc(s_pe, 1)

            # tiles 1-3 h0: classic double-buffered halves
            tensor.wait_ge(s_lta, 16)
            fill(1, range(4), 4, 2)
            tensor.wait_ge(s_act, 3)
            fill(2, range(4), 0, 2)
            tensor.wait_ge(s_act, 4)
            fill(3, range(4), 4, 2)
            # tile 4: pre-fill the low half during t3h0's read, then the
            # high half once t3h0's activation is done -- its FD-4096
            # read only touches the high banks ~1.9us after waking
            tensor.wait_ge(s_act, 5)
            tensor.wait_ge(s_ltb, 16)
            fill(4, range(4), 0, 2)
            tensor.wait_ge(s_act, 6)
            tensor.wait_ge(s_r1a, 16)
            tensor.wait_ge(s_r1b, 16)
            fill(4, range(4, 8), 4, None)
            # tiles 5-15: banks 0-6 are paced by the DVE behind the
            # previous tile's sweeping FD-4096 read (the shorter copy
            # keeps the pacer chain under the activation period); bank 7
            # waits until the chased activation has provably started
            # (its predecessor's READ_ACC completion), which is later
            # than that read's last touch of bank 7
            for k, t in enumerate(range(5, NT)):
                tensor.wait_ge(s_vd, k + 1)
                fill(t, range(6), 0, 2)
                tensor.wait_ge(s_act, t + 2)
                fill(t, [6, 7], 6, None)
            # h1 halves of tiles 3..0; the first chases tile 15's read,
            # the rest are semaphore-clean double-buffering
            tensor.wait_ge(s_vd, 12)
            fill(3, range(4, 8), 0, 2)
            tensor.wait_ge(s_act, 18)
            fill(2, range(4, 8), 4, 2)
            tensor.wait_ge(s_act, 19)
            fill(1, range(4, 8), 0, 2)
            tensor.wait_ge(s_act, 20)
            fill(0, range(4, 8), 4, 2)

    return nc


def _bf16_split2(x):
    import ml_dtypes

    bf = ml_dtypes.bfloat16
    x = x.astype(np.float32)
    p1 = x.astype(bf)
    p2 = (x - p1.astype(np.float32)).astype(bf)
    return p1, p2


def _bandwidth_np(X_fit):
    # mirror of reference._bandwidth (Silverman-style)
    b, n, d = X_fit.shape
    flat = np.asarray(X_fit, dtype=np.float64).reshape(-1)
    q = np.quantile(flat, 0.75) - np.quantile(flat, 0.25)
    std = np.std(np.asarray(X_fit, dtype=np.float64).reshape(b, -1), axis=1, ddof=1)
    return (0.9 * np.minimum(std, q / 1.34) / (n**0.2)).astype(np.float32)


def _host_prep(X_query, X_fit):
    import ml_dtypes

    bf = ml_dtypes.bfloat16
    X_query = np.asarray(X_query, dtype=np.float32)
    X_fit = np.asarray(X_fit, dtype=np.float32)
    bw = _bandwidth_np(X_fit)  # [B]

    in_maps = []
    for c in range(NCORES):
        b = c // SHARDS_PER_BATCH
        s = c % SHARDS_PER_BATCH
        XQ = X_query[b, s * NSHARD : (s + 1) * NSHARD]  # [2048, 32]
        XF = X_fit[b]  # [4096, 32]
        inv = np.float64(1.0) / np.float64(bw[b])

        # permuted queries: tile t / partition p handles query row p*NT+t;
        # 2/bw rides in the q pieces so psum IS the exp argument
        XQp = XQ.reshape(128, NT, D).transpose(1, 0, 2).reshape(NSHARD, D)
        Q = (2.0 * inv * XQp.T.astype(np.float64)).astype(np.float32)  # [32, 2048]
        q1, q2 = _bf16_split2(Q)
        FT = np.ascontiguousarray(XF.T.astype(np.float32))  # [32, 4096]
        f1, f2 = _bf16_split2(FT)
        sv = (FT.astype(np.float64) ** 2 * inv).sum(0).astype(np.float32)  # [4096]
        s1, s2 = _bf16_split2(sv)
        nx2 = (XQp.astype(np.float64) ** 2).sum(1)  # [2048] tile-major
        br = ((-nx2) * inv).astype(np.float32)
        b1, b2 = _bf16_split2(br)

        rhs = np.empty((KROWS, M), dtype=bf)
        rhs[0:32] = f1
        rhs[32:64] = f2
        rhs[64:96] = f1
        rhs[96] = s1
        rhs[97] = s2
        rhs[98:100] = np.ones((2, M), dtype=bf)

        blob = np.zeros((128, BLOB_W), dtype=bf)
        for t in range(NT):
            lo = _l_off(t)
            cols = slice(t * 128, (t + 1) * 128)
            blob[0:32, lo : lo + 128] = q1[:, cols]
            blob[32:64, lo : lo + 128] = q1[:, cols]
            blob[64:96, lo : lo + 128] = q2[:, cols]
            blob[96:98, lo : lo + 128] = -1.0
            blob[98, lo : lo + 128] = b1[cols]
            blob[99, lo : lo + 128] = b2[cols]
        blob[0:KROWS, OFF_R0:OFF_LT] = rhs[:, 0:2048]
        blob[0:KROWS, OFF_R1:BLOB_W] = rhs[:, 2048:4096]

        in_maps.append({"blob": blob})
    return in_maps


def _gather(results):
    out = np.empty((B, N), dtype=np.float32)
    for c in range(NCORES):
        b = c // SHARDS_PER_BATCH
        s = c % SHARDS_PER_BATCH
        res = np.asarray(results[c]["res"], dtype=np.float32)  # [128, 16]
        out[b, s * NSHARD : (s + 1) * NSHARD] = res.reshape(NSHARD)
    return out


def kernel(X_query, X_fit):
    from concourse.bass_utils import run_bass_kernel_spmd

    if "nc" not in _cached:
        _cached["nc"] = _build_program()
    nc = _cached["nc"]
    in_maps = _host_prep(X_query, X_fit)
    out = run_bass_kernel_spmd(nc, in_maps, list(range(NCORES)))
    return _gather(out.results)


# revision 30
# speedup vs baseline: 1.0166x; 1.0081x over previous
"""Batched KDE kernel for Trainium2 (8 NeuronCores, SPMD).

Problem: out[b, n] = sum_m exp(-||Xq[b,n] - Xf[b,m]||^2 / bw[b])
  with Silverman bandwidth bw[b] from Xf; b=4, n=m=4096, d=32.

Sharding: data-parallel over batch b (4 batches x 2 shards of query rows
= 8 cores). Each core handles n_shard=2048 query rows against the full
m=4096 fit set of its batch.

Device algorithm (per core), raw Bass with manual semaphores:
  The exp argument (2*q.f - |q|^2 - |f|^2)/bw is produced by ONE K=100
  bf16 matmul per 512-col psum bank (f32 values x = x1+x2+O(2^-16)):
    lhsT rows = [q1 q1 q2 | -1 -1 b1 b2]   (q pieces carry 2/bw)
    rhs  rows = [f1 f2 f1 | s1 s2 1  1]    (s = |f|^2/bw, b = -|q|^2/bw)
  capturing q1f1+q1f2+q2f1 (dropped q2f2 ~ 2e-3 on the argument,
  rel err ~4e-4 on the output -- tolerance is 2e-2).
  ScalarE activation computes exp(psum) with a fused per-partition
  accumulate (accum_out) -> the sum over m. ACT is the bottleneck
  (~1.94us per 2048-col group, 32 groups ~ 62us); PE at 1 matmul/chunk
  (~1us/group) runs far ahead, so the schedule is ACT-dense:
    - double-buffered psum (ps0/ps1, 4 banks each); ACT group g reads
      ps[g%2] while PE fills ps[(g+1)%2]; PE waits s_act >= g-1 before
      reusing a psum half
    - all constants (-1 rows, bias rows) are baked into the one dram
      blob; no memsets, no meta transfer, bias/scale are immediates
    - DMA is ordered by first use and split across both HWDGE queues:
      the first-group gate (lhsT t0 + rhs c0/c1 on sync, rhs c2/c3 on
      scalar) leads both in-order rings in parallel, and the bulk (lhsT
      t1-15, rhs chunks 4-7) is held behind explicit waits on the gate
      so it cannot crowd it out of the shared HBM port
    - s_pe for group g is released on its 3rd of 4 matmuls: the ACT
      reads bank 3 >=1.5us after waking while PE writes it ~0.25us
      after the release
    - PE warmup matmuls on garbage SBUF start immediately so the HAM
      clock gate is released before the first real group
    - group 0 runs as three pieces (512+512+1024 cols) so the first
      exp needs only lhsT t0 + rhs c0 (~130 KB) instead of the whole
      first-group gate
    - groups run h=0 tiles 0..15 then h=1 tiles 15..0 (reversed), so
      the final acc->res reduction is split in three and only a 3-slot
      reduce sits after the last exp group
  NOTE: engines run in relaxed ordering mode -- any same-engine RAW
  needs an explicit semaphore between producer and consumer.
Host does sharding/layout/packing plus the 4 scalar bandwidth values and
query norms (global quantile needs a sort; both are O(input) prep).
"""

import numpy as np

B, N, M, D = 4, 4096, 4096, 32
NCORES = 8
SHARDS_PER_BATCH = NCORES // B  # 2
NSHARD = N // SHARDS_PER_BATCH  # 2048
NT = NSHARD // 128  # 16 n-tiles per core
MCHUNK = 512  # matmul free-dim chunk (one psum bank)
ACT_FD = 2048  # activation free dim (4 psum banks)
NG = NT * (M // ACT_FD)  # 32 matmul/exp groups
KROWS = 100  # lhsT/rhs contraction rows

# blob column offsets (bf16 cols), ordered by first use
OFF_L0 = 0  # lhsT tile 0 (128)
OFF_R0 = 128  # rhs chunks 0-3 (2048)
OFF_LT = 2176  # lhsT tiles 1..15 (1920)
OFF_R1 = 4096  # rhs chunks 4-7 (2048)
BLOB_W = 6144

_cached = {}


def _l_off(t):
    return OFF_L0 if t == 0 else OFF_LT + (t - 1) * 128


def _r_off(c):  # c = m-col / 512, 0..7
    return OFF_R0 + c * MCHUNK if c < 4 else OFF_R1 + (c - 4) * MCHUNK


def _build_program():
    import concourse.bass as bass
    import concourse.mybir as mybir
    from contextlib import ExitStack

    nc = bass.Bass()
    f32 = mybir.dt.float32
    bf16 = mybir.dt.bfloat16

    blob = nc.declare_dram_parameter("blob", [128, BLOB_W], bf16, isOutput=False)
    res = nc.declare_dram_parameter("res", [128, NT], f32, isOutput=True)

    # pacer copy length: anchor (READ_ACC complete ~ T+240) + vec wake
    # (~100) + copy + inc/PE wake (~150) must land the fill start in
    # [T+2204, T+2839] where T = start of the activation being chased
    PACE_COLS = 5336  # copy ~1.45us; the DVE pipe DRAIN costs about a
    # full copy again, so copy must stay well under half the 3.65us
    # activation period or the pacer chain backs up

    with ExitStack() as ctx:
        msb = ctx.enter_context(nc.sbuf_tensor([128, BLOB_W], bf16))
        escr0 = ctx.enter_context(nc.sbuf_tensor([128, ACT_FD], bf16))
        escr1 = ctx.enter_context(nc.sbuf_tensor([128, ACT_FD], bf16))
        escr4 = ctx.enter_context(nc.sbuf_tensor([128, 2 * ACT_FD], bf16))
        # acc slots: group0 pieces 0,1,2; t0h1 3; tiles 1-3 -> 2+2t+h
        acc = ctx.enter_context(nc.sbuf_tensor([128, 10], f32))
        res_sb = ctx.enter_context(nc.sbuf_tensor([128, NT], f32))
        warmT = ctx.enter_context(nc.sbuf_tensor([128, 1], f32))
        wscr = ctx.enter_context(nc.sbuf_tensor([128, 640], bf16))
        vd1 = ctx.enter_context(nc.sbuf_tensor([128, PACE_COLS], bf16))
        vd2 = ctx.enter_context(nc.sbuf_tensor([128, PACE_COLS], bf16))
        psA = ctx.enter_context(nc.psum_tensor("psA", [128, 2 * ACT_FD], f32))

        s_l0 = ctx.enter_context(nc.semaphore("s_l0"))
        s_r0a = ctx.enter_context(nc.semaphore("s_r0a"))
        s_r0b = ctx.enter_context(nc.semaphore("s_r0b"))
        s_r1a = ctx.enter_context(nc.semaphore("s_r1a"))
        s_r1b = ctx.enter_context(nc.semaphore("s_r1b"))
        s_lta = ctx.enter_context(nc.semaphore("s_lta"))
        s_ltb = ctx.enter_context(nc.semaphore("s_ltb"))
        s_pe = ctx.enter_context(nc.semaphore("s_pe"))
        s_act = ctx.enter_context(nc.semaphore("s_act"))
        s_vd = ctx.enter_context(nc.semaphore("s_vd"))
        s_v3 = ctx.enter_context(nc.semaphore("s_v3"))
        sem_out = ctx.enter_context(nc.semaphore("sem_out"))
        block = ctx.enter_context(nc.Block())

        # ACT step table: (psum src, elementwise out, accum out) with
        # s_pe >= 1-based index gating each step.
        # idx 1-3: group-0 pieces (512+512+1024, tile 0 h0)
        # idx 4-6: tiles 1-3 h0 (FD 2048, alternating psum halves)
        # idx 7-18: tiles 4-15 full rows (FD 4096, accum -> res directly)
        # idx 19-22: tiles 3,2,1,0 h1 (FD 2048)
        lo, hi = psA[:, 0:ACT_FD], psA[:, ACT_FD : 2 * ACT_FD]
        steps = [
            (psA[:, 0:512], escr0[:, 0:512], acc[:, 0:1]),
            (psA[:, 512:1024], escr0[:, 0:512], acc[:, 1:2]),
            (psA[:, 1024:2048], escr0[:, 0:1024], acc[:, 2:3]),
            (hi, escr1[:], acc[:, 4:5]),
            (lo, escr0[:], acc[:, 6:7]),
            (hi, escr1[:], acc[:, 8:9]),
        ]
        for t in range(4, NT):
            steps.append((psA[:], escr4[:], res_sb[:, t : t + 1]))
        steps += [
            (lo, escr0[:], acc[:, 9:10]),
            (hi, escr1[:], acc[:, 7:8]),
            (lo, escr0[:], acc[:, 5:6]),
            (hi, escr1[:], acc[:, 3:4]),
        ]

        @block.sync
        def _(sync):
            # first-group gate leads both rings; bulk held behind it
            sync.dma_start(
                msb[0:KROWS, OFF_L0 : OFF_R0 + 512],
                blob[0:KROWS, OFF_L0 : OFF_R0 + 512],
            ).then_inc(s_l0, 16)
            sync.wait_ge(s_r0b, 16)
            sync.wait_ge(s_lta, 16)
            sync.dma_start(
                msb[0:KROWS, OFF_LT + 384 : OFF_R1],
                blob[0:KROWS, OFF_LT + 384 : OFF_R1],
            ).then_inc(s_ltb, 16)
            sync.dma_start(
                msb[0:KROWS, OFF_R1 : OFF_R1 + 1024],
                blob[0:KROWS, OFF_R1 : OFF_R1 + 1024],
            ).then_inc(s_r1a, 16)
            sync.dma_start(
                msb[0:KROWS, OFF_R1 + 1024 : BLOB_W],
                blob[0:KROWS, OFF_R1 + 1024 : BLOB_W],
            ).then_inc(s_r1b, 16)
            # tiles 4-15 land in res_sb straight from the accumulator
            # reads; step 18's READ_ACC fires s_act >= 18
            sync.wait_ge(s_act, 18)
            sync.dma_start(res[:, 4:16], res_sb[:, 4:16]).then_inc(sem_out, 16)
            sync.wait_ge(s_v3, 1)
            # no completion wait: the NEFF teardown drains the DMA queue
            sync.dma_start(res[:, 0:4], res_sb[:, 0:4]).then_inc(sem_out, 16)

        @block.vector
        def _(vector):
            # pacers: the fill for tile t (t=5..15) chases the FD-4096
            # read of tile t-1; t3h1's fill chases tile 15's read. Each
            # pacer anchors on the chased activation's start (its
            # predecessor's READ_ACC completion) and burns a
            # cycle-accurate DVE copy before releasing the PE.
            for k, a in enumerate(list(range(6, 17)) + [17]):
                vector.wait_ge(s_act, a)
                # the first paced fill consistently lands ~130ns late
                # (its anchor rides phase-1's cadence); shorten its copy
                cols = 4700 if k == 0 else PACE_COLS
                nc.vector.tensor_copy(
                    vd2[:, 0:cols], vd1[:, 0:cols]
                ).then_inc(s_vd, 1)
            # tail reductions for the h-split tiles, earliest first
            vector.wait_ge(s_act, 19)
            nc.vector.tensor_reduce(
                res_sb[:, 3:4],
                acc[:, 8:10].rearrange("p (t h) -> p t h", h=2),
                axis=mybir.AxisListType.X,
                op=mybir.AluOpType.add,
            )
            vector.wait_ge(s_act, 20)
            nc.vector.tensor_reduce(
                res_sb[:, 2:3],
                acc[:, 6:8].rearrange("p (t h) -> p t h", h=2),
                axis=mybir.AxisListType.X,
                op=mybir.AluOpType.add,
            )
            vector.wait_ge(s_act, 21)
            nc.vector.tensor_reduce(
                res_sb[:, 1:2],
                acc[:, 4:6].rearrange("p (t h) -> p t h", h=2),
                axis=mybir.AxisListType.X,
                op=mybir.AluOpType.add,
            )
            vector.wait_ge(s_act, 22)
            nc.vector.tensor_reduce(
                res_sb[:, 0:1],
                acc[:, 0:4].rearrange("p (t h) -> p t h", h=4),
                axis=mybir.AxisListType.X,
                op=mybir.AluOpType.add,
            ).then_inc(s_v3, 1)

        @block.scalar
        def _(scalar):
            # fire the exp table-set load FIRST: it takes ~1.6us and
            # otherwise gates the first exp from behind the DMA issues
            # (the psum data is ready before the table when it is last)
            nc.scalar.activation(
                warmT[:], warmT[:], mybir.ActivationFunctionType.Exp,
                bias=res_sb[:, 0:1],
            )
            # second HWDGE queue: rhs c1, c2/c3, then lhsT tiles 1-3
            scalar.dma_start(
                msb[0:KROWS, OFF_R0 + 512 : OFF_R0 + 1024],
                blob[0:KROWS, OFF_R0 + 512 : OFF_R0 + 1024],
            ).then_inc(s_r0a, 16)
            scalar.dma_start(
                msb[0:KROWS, OFF_R0 + 1024 : OFF_LT],
                blob[0:KROWS, OFF_R0 + 1024 : OFF_LT],
            ).then_inc(s_r0b, 16)
            scalar.dma_start(
                msb[0:KROWS, OFF_LT : OFF_LT + 384],
                blob[0:KROWS, OFF_LT : OFF_LT + 384],
            ).then_inc(s_lta, 16)
            for k, (src_ap, out_ap, acc_ap) in enumerate(steps):
                scalar.wait_ge(s_pe, k + 1)
                nc.scalar.add_instruction(
                    mybir.InstActivation(
                        name=nc.get_next_instruction_name(),
                        func=mybir.ActivationFunctionType.Exp,
                        ins=[
                            nc.scalar.lower_ap(src_ap),
                            mybir.ImmediateValue(
                                dtype=mybir.dt.float32, value=0.0
                            ),
                            mybir.ImmediateValue(
                                dtype=mybir.dt.float32, value=1.0
                            ),
                            mybir.ImmediateValue(
                                dtype=mybir.dt.float32, value=0.0
                            ),
                        ],
                        outs=[
                            nc.scalar.lower_ap(out_ap),
                            nc.scalar.lower_ap(acc_ap),
                        ],
                    )
                ).then_inc(s_act, 1)

        @block.tensor
        def _(tensor):
            # warm the PE clock (HAM) with dummy matmuls on garbage SBUF;
            # they run at half clock (~427ns each) so keep them few: the
            # first-piece matmuls have 2-3x slack vs their activations
            # even if the clock is still ramping
            for _w in range(10):
                nc.tensor.matmul(
                    psA[:, 0:MCHUNK],
                    wscr[:, 0:128],
                    wscr[:, 128:640],
                    start=True,
                    stop=True,
                )

            def fill(t, chunks, bank0, rel_on, sub=None):
                # one matmul per 512-col bank; inc s_pe on matmul rel_on
                lh = msb[0:KROWS, _l_off(t) : _l_off(t) + 128]
                for j, c in enumerate(chunks):
                    if sub:
                        sub(j)
                    mm = nc.tensor.matmul(
                        psA[:, (bank0 + j) * MCHUNK : (bank0 + j + 1) * MCHUNK],
                        lh,
                        msb[0:KROWS, _r_off(c) : _r_off(c) + MCHUNK],
                        start=True,
                        stop=True,
                    )
                    if j == rel_on:
                        mm.then_inc(s_pe, 1)

            # group-0 pieces: release per piece as its banks complete
            tensor.wait_ge(s_l0, 16)

            def g0_sub(j):
                if j == 1:
                    tensor.wait_ge(s_r0a, 16)
                if j == 2:
                    tensor.wait_ge(s_r0b, 16)

            lh0 = msb[0:KROWS, _l_off(0) : _l_off(0) + 128]
            for j in range(4):
                g0_sub(j)
                mm = nc.tensor.matmul(
                    psA[:, j * MCHUNK : (j + 1) * MCHUNK],
                    lh0,
                    msb[0:KROWS, _r_off(j) : _r_off(j) + MCHUNK],
                    start=True,
                    stop=True,
                )
                if j in (0, 1, 3):
                    mm.then_inc(s_pe, 1)

            # tiles 1-3 h0: classic double-buffered halves
            tensor.wait_ge(s_lta, 16)
            fill(1, range(4), 4, 2)
            tensor.wait_ge(s_act, 3)
            fill(2, range(4), 0, 2)
            tensor.wait_ge(s_act, 4)
            fill(3, range(4), 4, 2)
            # tile 4: pre-fill the low half during t3h0's read, then the
            # high half once t3h0's activation is done -- its FD-4096
            # read only touches the high banks ~1.9us after waking
            tensor.wait_ge(s_act, 5)
            tensor.wait_ge(s_ltb, 16)
            fill(4, range(4), 0, 2)
            tensor.wait_ge(s_act, 6)
            tensor.wait_ge(s_r1a, 16)
            tensor.wait_ge(s_r1b, 16)
            fill(4, range(4, 8), 4, None)
            # tiles 5-15: banks 0-6 are paced by the DVE behind the
            # previous tile's sweeping FD-4096 read (the shorter copy
            # keeps the pacer chain under the activation period); bank 7
            # waits until the chased activation has provably started
            # (its predecessor's READ_ACC completion), which is later
            # than that read's last touch of bank 7
            for k, t in enumerate(range(5, NT)):
                tensor.wait_ge(s_vd, k + 1)
                fill(t, range(6), 0, 2)
                tensor.wait_ge(s_act, t + 2)
                fill(t, [6, 7], 6, None)
            # h1 halves of tiles 3..0; the first chases tile 15's read,
            # the rest are semaphore-clean double-buffering
            tensor.wait_ge(s_vd, 12)
            fill(3, range(4, 8), 0, 2)
            tensor.wait_ge(s_act, 18)
            fill(2, range(4, 8), 4, 2)
            tensor.wait_ge(s_act, 19)
            fill(1, range(4, 8), 0, 2)
            tensor.wait_ge(s_act, 20)
            fill(0, range(4, 8), 4, 2)

    return nc


def _bf16_split2(x):
    import ml_dtypes

    bf = ml_dtypes.bfloat16
    x = x.astype(np.float32)
    p1 = x.astype(bf)
    p2 = (x - p1.astype(np.float32)).astype(bf)
    return p1, p2


def _bandwidth_np(X_fit):
    # mirror of reference._bandwidth (Silverman-style)
    b, n, d = X_fit.shape
    flat = np.asarray(X_fit, dtype=np.float64).reshape(-1)
    q = np.quantile(flat, 0.75) - np.quantile(flat, 0.25)
    std = np.std(np.asarray(X_fit, dtype=np.float64).reshape(b, -1), axis=1, ddof=1)
    return (0.9 * np.minimum(std, q / 1.34) / (n**0.2)).astype(np.float32)


def _host_prep(X_query, X_fit):
    import ml_dtypes

    bf = ml_dtypes.bfloat16
    X_query = np.asarray(X_query, dtype=np.float32)
    X_fit = np.asarray(X_fit, dtype=np.float32)
    bw = _bandwidth_np(X_fit)  # [B]

    in_maps = []
    for c in range(NCORES):
        b = c // SHARDS_PER_BATCH
        s = c % SHARDS_PER_BATCH
        XQ = X_query[b, s * NSHARD : (s + 1) * NSHARD]  # [2048, 32]
        XF = X_fit[b]  # [4096, 32]
        inv = np.float64(1.0) / np.float64(bw[b])

        # permuted queries: tile t / partition p handles query row p*NT+t;
        # 2/bw rides in the q pieces so psum IS the exp argument
        XQp = XQ.reshape(128, NT, D).transpose(1, 0, 2).reshape(NSHARD, D)
        Q = (2.0 * inv * XQp.T.astype(np.float64)).astype(np.float32)  # [32, 2048]
        q1, q2 = _bf16_split2(Q)
        FT = np.ascontiguousarray(XF.T.astype(np.float32))  # [32, 4096]
        f1, f2 = _bf16_split2(FT)
        sv = (FT.astype(np.float64) ** 2 * inv).sum(0).astype(np.float32)  # [4096]
        s1, s2 = _bf16_split2(sv)
        nx2 = (XQp.astype(np.float64) ** 2).sum(1)  # [2048] tile-major
        br = ((-nx2) * inv).astype(np.float32)
        b1, b2 = _bf16_split2(br)

        rhs = np.empty((KROWS, M), dtype=bf)
        rhs[0:32] = f1
        rhs[32:64] = f2
        rhs[64:96] = f1
        rhs[96] = s1
        rhs[97] = s2
        rhs[98:100] = np.ones((2, M), dtype=bf)

        blob = np.zeros((128, BLOB_W), dtype=bf)
        for t in range(NT):
            lo = _l_off(t)
            cols = slice(t * 128, (t + 1) * 128)
            blob[0:32, lo : lo + 128] = q1[:, cols]
            blob[32:64, lo : lo + 128] = q1[:, cols]
            blob[64:96, lo : lo + 128] = q2[:, cols]
            blob[96:98, lo : lo + 128] = -1.0
            blob[98, lo : lo + 128] = b1[cols]
            blob[99, lo : lo + 128] = b2[cols]
        blob[0:KROWS, OFF_R0:OFF_LT] = rhs[:, 0:2048]
        blob[0:KROWS, OFF_R1:BLOB_W] = rhs[:, 2048:4096]

        in_maps.append({"blob": blob})
    return in_maps


def _gather(results):
    out = np.empty((B, N), dtype=np.float32)
    for c in range(NCORES):
        b = c // SHARDS_PER_BATCH
        s = c % SHARDS_PER_BATCH
        res = np.asarray(results[c]["res"], dtype=np.float32)  # [128, 16]
        out[b, s * NSHARD : (s + 1) * NSHARD] = res.reshape(NSHARD)
    return out


def kernel(X_query, X_fit):
    from concourse.bass_utils import run_bass_kernel_spmd

    if "nc" not in _cached:
        _cached["nc"] = _build_program()
    nc = _cached["nc"]
    in_maps = _host_prep(X_query, X_fit)
    out = run_bass_kernel_spmd(nc, in_maps, list(range(NCORES)))
    return _gather(out.results)
